# revision 1
# baseline (speedup 1.0000x reference)
"""Child-Sum TreeLSTM (complete binary trees) on 8 TRN2 NeuronCores.

Problem: B=256 trees, N=511 nodes (depth 9), D_IN=300, H=512.
Sharding: data-parallel over trees -- 32 trees per core, weights replicated.

Device algorithm (per core, all 32 trees):
  One uniform bottom-up level loop (level 8 = leaves ... level 0 = root).
  Everything is kept feature-on-partition ("transposed") so no on-device
  transposes are needed:
    x^T   [300pad384, cols]  (host pre-transposed, level-major column order)
    h^T/c^T per level [512=4x128, cols]  col = tree*2^l + node
  With tree-major column order, the children of parent column j are child
  columns 2j, 2j+1 of the level below -- child-sum and f*c reductions are
  stride-2 vector ops, and each parent chunk [p0,p0+P) consumes exactly
  child columns [2p0, 2p0+2P).

  Per level, per 512-column chunk:
    iou^T accumulated in PSUM: W_ioux^T x^T (3 K-chunks) + W_iouh^T hsum^T
    (4 K-chunks), evacuated by ACT with fused bias+sigmoid/tanh.
    f^T for both children in one go over child columns: W_fh^T h_child^T
    + W_fx^T x_dup^T (parent x duplicated via 0-step AP), sigmoid evac.
    c = sig(i)*tanh(u) + f_even*c_even + f_odd*c_odd ; h = sig(o)*tanh(c).
  h,c level state is bounced through DRAM scratch (SBUF can't hold the
  leaf levels); chunk-granular dependencies let Tile pipeline levels.

  Matmuls run as float32r (full-rate fp32, ~1e-4 rel err on TRN2).
"""

import sys

sys.path.insert(0, "/opt/trn_rl_repo")

from contextlib import ExitStack

import numpy as np

import concourse.bass as bass
import concourse.tile as tile
from concourse import bacc, mybir

F32 = mybir.dt.float32
F32R = mybir.dt.float32r
BF16 = mybir.dt.bfloat16
AFT = mybir.ActivationFunctionType

B, NTREE, DIN, H = 256, 511, 300, 512
NCORES = 8
BC = B // NCORES  # 32 trees per core
DEPTH = 9
KX = 3  # K chunks for D_IN (300 -> 3*128 padded)
KH = 4  # K chunks for H (512 = 4*128)
LCOLS = [BC * (1 << l) for l in range(DEPTH)]  # cols per level, index=level
TOTCOLS = sum(LCOLS)  # 16352
# column offset of each level in x^T (level-major, descending level)
LOFF = {}
_off = 0
for _l in range(DEPTH - 1, -1, -1):
    LOFF[_l] = _off
    _off += LCOLS[_l]

CHUNK = 512
# chunk-major layout tables: per level (desc), chunk width + number of chunks
PW = {l: min(CHUNK, LCOLS[l]) for l in range(DEPTH)}
NCH = {l: (LCOLS[l] + PW[l] - 1) // PW[l] for l in range(DEPTH)}
# global chunk index base per level for the x^T slab
CIBASE = {}
_ci = 0
for _l in range(DEPTH - 1, -1, -1):
    CIBASE[_l] = _ci
    _ci += NCH[_l]
NCHTOT = _ci  # 35


def build_program():
    nc = bacc.Bacc("TRN2", target_bir_lowering=False, debug=False)

    d_xt = nc.dram_tensor("xt", [NCHTOT, 128, KX * CHUNK], BF16, kind="ExternalInput").ap()
    d_wioux = nc.dram_tensor("wioux", [KX, 128, 3 * H], BF16, kind="ExternalInput").ap()
    d_wiouh = nc.dram_tensor("wiouh", [KH, 128, 3 * H], BF16, kind="ExternalInput").ap()
    d_wfx = nc.dram_tensor("wfx", [KX, 128, H], BF16, kind="ExternalInput").ap()
    d_wfh = nc.dram_tensor("wfh", [KH, 128, H], BF16, kind="ExternalInput").ap()
    d_biou = nc.dram_tensor("biou", [128, 12], F32, kind="ExternalInput").ap()
    d_bf = nc.dram_tensor("bf", [128, 4], F32, kind="ExternalInput").ap()

    d_cout = nc.dram_tensor("c_out", [128, 4, BC], F32, kind="ExternalOutput").ap()
    d_hout = nc.dram_tensor("h_out", [128, 4, BC], F32, kind="ExternalOutput").ap()

    with tile.TileContext(nc) as tc, ExitStack() as ctx:
        wpool = ctx.enter_context(tc.tile_pool(name="weights", bufs=1))
        xpool = ctx.enter_context(tc.tile_pool(name="x", bufs=4))
        hchp = ctx.enter_context(tc.tile_pool(name="hch", bufs=4))
        cchp = ctx.enter_context(tc.tile_pool(name="cch", bufs=4))
        outp = ctx.enter_context(tc.tile_pool(name="state", bufs=4))
        workp = ctx.enter_context(tc.tile_pool(name="work", bufs=3))
        fcp = ctx.enter_context(tc.tile_pool(name="fc", bufs=2))
        hsump = ctx.enter_context(tc.tile_pool(name="hsum", bufs=5))
        psump = ctx.enter_context(tc.tile_pool(name="psum", bufs=8, space="PSUM"))
        dramp = ctx.enter_context(tc.tile_pool(name="dram", bufs=1, space="DRAM"))

        # ---- weights / biases (one-time casting DMAs to f32r) ----
        s_wioux = wpool.tile([128, KX, 3 * H], BF16)
        s_wiouh = wpool.tile([128, KH, 3 * H], BF16)
        s_wfx = wpool.tile([128, KX, H], BF16)
        s_wfh = wpool.tile([128, KH, H], BF16)
        for k in range(KX):
            nc.sync.dma_start(out=s_wioux[:, k, :], in_=d_wioux[k])
            nc.sync.dma_start(out=s_wfx[:, k, :], in_=d_wfx[k])
        for k in range(KH):
            nc.sync.dma_start(out=s_wiouh[:, k, :], in_=d_wiouh[k])
            nc.sync.dma_start(out=s_wfh[:, k, :], in_=d_wfh[k])
        s_biou = wpool.tile([128, 12], F32)
        s_bf = wpool.tile([128, 4], F32)
        nc.sync.dma_start(out=s_biou, in_=d_biou)
        nc.sync.dma_start(out=s_bf, in_=d_bf)

        # ---- DRAM scratch for per-level h/c state (levels 8..1) ----
        hd = {}
        cd = {}
        for l in range(1, DEPTH):
            if NCH[l] <= 2:
                continue  # small levels stay SBUF-resident
            hd[l] = dramp.tile([NCH[l], 128, KH, PW[l]], BF16, tag=f"hd{l}", name=f"hd{l}")
            cd[l] = dramp.tile([NCH[l], 128, KH, PW[l]], F32, tag=f"cd{l}", name=f"cd{l}")

        def dup_ap(base):
            """Each column of `base` twice: [128, W] -> [128, W, 2] (0-step)."""
            return bass.AP(
                tensor=base.tensor,
                offset=base.offset,
                ap=list(base.ap) + [[0, 2]],
            )

        def iou_psum(m, P, xt, hsum):
            """PSUM accumulation for iou feature chunk m over P cols."""
            ps = psump.tile([128, CHUNK], F32, tag="ps")
            last_x = hsum is None
            for k in range(KX):
                nc.tensor.matmul(
                    ps[:, :P],
                    s_wioux[:, k, 128 * m : 128 * m + 128],
                    xt[:, k, :P],
                    start=(k == 0),
                    stop=(last_x and k == KX - 1),
                )
            if hsum is not None:
                for k in range(KH):
                    nc.tensor.matmul(
                        ps[:, :P],
                        s_wiouh[:, k, 128 * m : 128 * m + 128],
                        hsum[:, k, :P],
                        start=False,
                        stop=(k == KH - 1),
                    )
            return ps

        sbuf_child = {}  # level -> (list of (ht, ct) per chunk, chunk width)

        def process_level(l):
            C = LCOLS[l]
            P = min(CHUNK, C)
            is_leaf = l == DEPTH - 1
            small = C <= CHUNK  # latency-optimized tail levels
            for j in range(0, C, P):
                # x^T chunk [128, KX, 512] -- one contiguous chunk-major load
                ci = CIBASE[l] + j // P
                xt = xpool.tile([128, KX, CHUNK], BF16, tag="xt")
                nc.sync.dma_start(
                    out=xt[:].rearrange("p k c -> p (k c)"), in_=d_xt[ci]
                )

                ct = outp.tile([128, KH, CHUNK], F32, tag="ct")
                ht = outp.tile([128, KH, CHUNK], BF16, tag="ht")

                hsum = None
                if not is_leaf:
                    # children: cols [2j, 2j+2P) of level l+1 = child chunks
                    if l + 1 in sbuf_child:
                        tiles, pw = sbuf_child[l + 1]
                        npieces = 2 * P // pw
                        cj0 = 2 * j // pw
                        hch = [tiles[cj0 + pc][0] for pc in range(npieces)]
                        cch = [tiles[cj0 + pc][1] for pc in range(npieces)]
                    else:
                        pw = PW[l + 1]
                        npieces = 2 * P // pw
                        cj0 = 2 * j // pw
                        hch, cch = [], []
                        for pc in range(npieces):
                            hc = hchp.tile([128, KH, CHUNK], BF16, tag="hch")
                            cc = cchp.tile([128, KH, CHUNK], F32, tag="cch")
                            nc.sync.dma_start(out=hc[:, :, :pw], in_=hd[l + 1][cj0 + pc])
                            nc.sync.dma_start(out=cc[:, :, :pw], in_=cd[l + 1][cj0 + pc])
                            hch.append(hc)
                            cch.append(cc)

                    # hsum[:, :, a:a+pw/2] = hch[...,0::2] + [...,1::2]
                    hsum = hsump.tile([128, KH, CHUNK], BF16, tag="hsum")
                    for pc in range(npieces):
                        pair = hch[pc][:, :, :pw].rearrange(
                            "p k (n two) -> p k n two", two=2
                        )
                        a = pc * (pw // 2)
                        heng = nc.vector if small else nc.gpsimd
                        heng.tensor_add(
                            out=hsum[:, :, a : a + pw // 2],
                            in0=pair[:, :, :, 0],
                            in1=pair[:, :, :, 1],
                        )

                # ---- i/u gates: c = sigmoid(i) * tanh(u) ----
                for m in range(4):
                    ps_u = iou_psum(8 + m, P, xt, hsum)
                    tu = workp.tile([128, CHUNK], F32, tag="tu")
                    nc.scalar.activation(
                        tu[:, :P], ps_u[:, :P], AFT.Tanh, bias=s_biou[:, 8 + m : 9 + m]
                    )
                    ps_i = iou_psum(m, P, xt, hsum)
                    nc.scalar.activation(
                        ct[:, m, :P], ps_i[:, :P], AFT.Sigmoid, bias=s_biou[:, m : m + 1]
                    )
                    nc.vector.tensor_mul(ct[:, m, :P], ct[:, m, :P], tu[:, :P])

                # ---- forget gates + fc accumulation into c ----
                if not is_leaf:
                    for m in range(4):
                        fc = fcp.tile([128, 2 * CHUNK], F32, tag="fc")
                        if small:
                            # latency-optimized: fx accumulated in PSUM (dup-AP)
                            for pc in range(npieces):
                                s = pc * pw
                                ps = psump.tile([128, CHUNK], F32, tag="ps")
                                for k in range(KH):
                                    nc.tensor.matmul(
                                        ps[:, :pw],
                                        s_wfh[:, k, 128 * m : 128 * m + 128],
                                        hch[pc][:, k, :pw],
                                        start=(k == 0),
                                        stop=False,
                                    )
                                for k in range(KX):
                                    xb = xt[:, k, s // 2 : s // 2 + pw // 2]
                                    nc.tensor.matmul(
                                        ps[:, :pw],
                                        s_wfx[:, k, 128 * m : 128 * m + 128],
                                        dup_ap(xb),
                                        start=False,
                                        stop=(k == KX - 1),
                                    )
                                f = workp.tile([128, CHUNK], F32, tag="f")
                                nc.scalar.activation(
                                    f[:, :pw], ps[:, :pw], AFT.Sigmoid,
                                    bias=s_bf[:, m : m + 1],
                                )
                                nc.vector.tensor_mul(
                                    fc[:, s : s + pw], f[:, :pw], cch[pc][:, m, :pw]
                                )
                        else:
                            # fx for parent cols, with the forget bias folded in
                            ps_fx = psump.tile([128, CHUNK], F32, tag="ps")
                            for k in range(KX):
                                nc.tensor.matmul(
                                    ps_fx[:, :P],
                                    s_wfx[:, k, 128 * m : 128 * m + 128],
                                    xt[:, k, :P],
                                    start=(k == 0),
                                    stop=(k == KX - 1),
                                )
                            fx = workp.tile([128, CHUNK], F32, tag="fx")
                            nc.scalar.activation(
                                fx[:, :P], ps_fx[:, :P], AFT.Identity, scale=1.0,
                                bias=s_bf[:, m : m + 1],
                            )
                            pss = [psump.tile([128, CHUNK], F32, tag="ps", name=f"psf{pc}") for pc in range(npieces)]
                            for k in range(KH):  # k-outer: consecutive MMs share lhsT
                                for pc in range(npieces):
                                    nc.tensor.matmul(
                                        pss[pc][:, :pw],
                                        s_wfh[:, k, 128 * m : 128 * m + 128],
                                        hch[pc][:, k, :pw],
                                        start=(k == 0),
                                        stop=(k == KH - 1),
                                    )
                            for pc in range(npieces):
                                s = pc * pw
                                # f_pre = fh_psum + fx(parent, duplicated via 0-step)
                                f = workp.tile([128, CHUNK], F32, tag="f")
                                nc.vector.scalar_tensor_tensor(
                                    out=f[:, :pw],
                                    in0=pss[pc][:, :pw],
                                    scalar=1.0,
                                    in1=dup_ap(fx[:, s // 2 : s // 2 + pw // 2]),
                                    op0=mybir.AluOpType.mult,
                                    op1=mybir.AluOpType.add,
                                )
                                nc.scalar.activation(f[:, :pw], f[:, :pw], AFT.Sigmoid)
                                nc.vector.tensor_mul(
                                    fc[:, s : s + pw], f[:, :pw], cch[pc][:, m, :pw]
                                )
                        fpair = fc[:, : 2 * P].rearrange("p (n two) -> p n two", two=2)
                        aeng = nc.vector if small else nc.gpsimd
                        aeng.tensor_add(
                            out=ct[:, m, :P], in0=ct[:, m, :P], in1=fpair[:, :, 0]
                        )
                        aeng.tensor_add(
                            out=ct[:, m, :P], in0=ct[:, m, :P], in1=fpair[:, :, 1]
                        )

                # ---- o gates, h = sigmoid(o) * tanh(c) ----
                if small:
                    # latency-optimized per-m chain
                    for m in range(4):
                        ps_o = iou_psum(4 + m, P, xt, hsum)
                        so = workp.tile([128, CHUNK], F32, tag="so")
                        nc.scalar.activation(
                            so[:, :P], ps_o[:, :P], AFT.Sigmoid,
                            bias=s_biou[:, 4 + m : 5 + m],
                        )
                        tc_ = workp.tile([128, CHUNK], F32, tag="tc")
                        nc.scalar.activation(tc_[:, :P], ct[:, m, :P], AFT.Tanh)
                        nc.vector.tensor_mul(ht[:, m, :P], so[:, :P], tc_[:, :P])
                else:
                    # throughput-optimized: bf16 so/tanh tiles, fused wide ops
                    so4 = hsump.tile([128, KH, CHUNK], BF16, tag="hsum", name="so4")
                    for m in range(4):
                        ps_o = iou_psum(4 + m, P, xt, hsum)
                        nc.scalar.activation(
                            so4[:, m, :P], ps_o[:, :P], AFT.Sigmoid,
                            bias=s_biou[:, 4 + m : 5 + m],
                        )
                    tc4 = hsump.tile([128, KH, CHUNK], BF16, tag="hsum", name="tc4")
                    nc.scalar.activation(
                        tc4[:].rearrange("p k c -> p (k c)"),
                        ct[:].rearrange("p k c -> p (k c)"),
                        AFT.Tanh,
                    )
                    nc.vector.tensor_mul(
                        ht[:].rearrange("p k c -> p (k c)"),
                        so4[:].rearrange("p k c -> p (k c)"),
                        tc4[:].rearrange("p k c -> p (k c)"),
                    )

                # ---- store state ----
                if l == 0:
                    nc.gpsimd.dma_start(out=d_hout, in_=ht[:, :, :P])
                    nc.sync.dma_start(out=d_cout, in_=ct[:, :, :P])
                elif NCH[l] <= 2:
                    # small level: children consumed straight from SBUF
                    sbuf_child.setdefault(l, ([], P))[0].append((ht, ct))
                else:
                    nc.sync.dma_start(out=hd[l][j // P], in_=ht[:, :, :P])
                    nc.sync.dma_start(out=cd[l][j // P], in_=ct[:, :, :P])

        for l in range(DEPTH - 1, -1, -1):
            process_level(l)

    nc.compile()
    return nc


_nc_cache = None


def get_program():
    global _nc_cache
    if _nc_cache is None:
        _nc_cache = build_program()
    return _nc_cache


def prep_inputs(inputs, W_ioux, b_ioux, W_iouh, b_iouh, W_fx, b_fx, W_fh, b_fh):
    """Host-side prep: per-core x^T slabs + padded weight chunks + fused biases."""
    inputs = np.ascontiguousarray(np.asarray(inputs, dtype=np.float32))

    import ml_dtypes

    BF = ml_dtypes.bfloat16

    def padk(w, rows):
        w = np.asarray(w, np.float32)
        out = np.zeros((rows * 128, w.shape[1]), np.float32)
        out[: w.shape[0]] = w
        return np.ascontiguousarray(out.reshape(rows, 128, w.shape[1]).astype(BF))

    wioux = padk(W_ioux, KX)
    wiouh = padk(W_iouh, KH)
    wfx = padk(W_fx, KX)
    wfh = padk(W_fh, KH)
    biou = np.ascontiguousarray(
        (np.asarray(b_ioux) + np.asarray(b_iouh)).astype(np.float32).reshape(12, 128).T
    )
    bf = np.ascontiguousarray(
        (np.asarray(b_fx) + np.asarray(b_fh)).astype(np.float32).reshape(4, 128).T
    )

    in_maps = []
    for c in range(NCORES):
        xc = inputs[c * BC : (c + 1) * BC]  # [BC, NTREE, DIN]
        blocks = []
        for l in range(DEPTH - 1, -1, -1):
            a, b = (1 << l) - 1, (1 << (l + 1)) - 1
            blocks.append(xc[:, a:b, :].reshape(-1, DIN))  # tree-major
        xcols = np.concatenate(blocks, axis=0)  # [TOTCOLS, DIN]
        xtf = np.zeros((KX * 128, TOTCOLS), np.float32)
        xtf[:DIN] = xcols.T
        xtf = xtf.reshape(KX, 128, TOTCOLS)
        xt = np.zeros((NCHTOT, 128, KX, CHUNK), np.float32)
        for l in range(DEPTH - 1, -1, -1):
            for jj in range(NCH[l]):
                a = LOFF[l] + jj * PW[l]
                xt[CIBASE[l] + jj, :, :, : PW[l]] = xtf[:, :, a : a + PW[l]].transpose(
                    1, 0, 2
                )
        xt = np.ascontiguousarray(xt.reshape(NCHTOT, 128, KX * CHUNK).astype(BF))
        in_maps.append(
            {
                "xt": xt,
                "wioux": wioux,
                "wiouh": wiouh,
                "wfx": wfx,
                "wfh": wfh,
                "biou": biou,
                "bf": bf,
            }
        )
    return in_maps


def assemble_output(results):
    """results: list of per-core dicts with c_out/h_out [128, 4, BC]."""
    cs, hs = [], []
    for r in results:
        # [128part=feat%128, m=feat//128, tree] -> [tree, 512]
        c = np.transpose(r["c_out"], (2, 1, 0)).reshape(BC, H)
        h = np.transpose(r["h_out"], (2, 1, 0)).reshape(BC, H)
        cs.append(c)
        hs.append(h)
    return np.concatenate(cs, 0), np.concatenate(hs, 0)


def run_on_hw(in_maps, trace=False, tmpdir=None):
    from concourse.bass_utils import run_bass_kernel_spmd

    nc = get_program()
    return run_bass_kernel_spmd(
        nc, in_maps, list(range(NCORES)), trace=trace, tmpdir=tmpdir
    )


def kernel(**inputs):
    in_maps = prep_inputs(**inputs)
    res = run_on_hw(in_maps)
    return assemble_output(res.results)



# revision 2
# speedup vs baseline: 1.0733x; 1.0733x over previous
"""Child-Sum TreeLSTM (complete binary trees) on 8 TRN2 NeuronCores — v2.

Problem: B=256 trees, N=511 nodes (depth 9), D_IN=300, H=512.
Sharding: data-parallel over trees -- 32 trees per core, weights replicated.

v2 design (vs the DRAM-bounce baseline):
  * Tree-group blocking: the 32 trees are processed as 4 groups of 8.
    Each group's levels 8..5 fit entirely in SBUF (peak state ~10MB), so
    h/c never round-trips through DRAM.  Levels 4..0 run merged over all
    32 trees.  Group g+1's leaf work overlaps group g's narrow tail.
  * Split child layout: within a group, level l+1 columns are stored
    [left-children(parent order) | right-children(parent order)].  Child-sum,
    f*c and the fx broadcast all become contiguous (packed) vector ops,
    which unlocks the DVE 4x bf16 mode.  Levels <=3 use the baseline
    interleaved stride-2 scheme (tiny widths).
  * Biases ride in the x GEMM: x is augmented with two constant-1 rows
    (300, 301) and the bias (split hi+lo for fp8 rounding) is placed in the
    matching weight rows.  All ACT evacuations become wide 4-m-chunk ops
    with no bias APs.
  * fp8 (e4m3) DoubleRow matmuls at levels 8 and 7 (2x PE rate for the
    K=512 h-GEMMs, 1.5x for the K=384 x-GEMMs).  Measured end-to-end
    rel-err ~1.7e-2 in emulation (gate is 2e-2); everything else bf16.
  * All state (h, c) and intermediates in bf16; PSUM f32.
"""

import sys

sys.path.insert(0, "/opt/trn_rl_repo")

from contextlib import ExitStack

import numpy as np

import concourse.bass as bass
import concourse.tile as tile
from concourse import bacc, mybir

F32 = mybir.dt.float32
BF16 = mybir.dt.bfloat16
FP8 = mybir.dt.float8e4
AFT = mybir.ActivationFunctionType
DR = mybir.MatmulPerfMode.DoubleRow

B, NTREE, DIN, H = 256, 511, 300, 512
NCORES = 8
BC = B // NCORES  # 32 trees per core
DEPTH = 9
KX = 3  # K chunks for D_IN+bias rows (302 -> 3*128)
KH = 4  # K chunks for H (512 = 4*128)
T = 8  # trees per group
G = 4  # groups per core

FP8_LEVELS = {8, 7}

# per-group level widths (levels 8..5), merged widths (4..0)
GW = {8: T * 256, 7: T * 128, 6: T * 64, 5: T * 32}  # 2048,1024,512,256
MW = {4: BC * 16, 3: BC * 8, 2: BC * 4, 1: BC * 2, 0: BC}  # 512..32
CHUNK = 512
NCH_G = {lvl: max(1, GW[lvl] // CHUNK) for lvl in GW}  # 4,2,1,1

# ---- column orders (split layout in groups, interleaved merged) ----


def _build_orders():
    ords = {}  # (lvl, g) -> list[(tree, node)], g=None for merged
    for lvl in range(5):
        ords[(lvl, None)] = [(t, n) for t in range(BC) for n in range(1 << lvl)]
    base4 = ords[(4, None)]
    for g in range(G):
        blk = base4[g * 128 : (g + 1) * 128]  # group g's L4 cols (t-major)
        cur = [(t, 2 * n) for (t, n) in blk] + [(t, 2 * n + 1) for (t, n) in blk]
        ords[(5, g)] = cur
        for lvl in range(6, DEPTH):
            cur = [(t, 2 * n) for (t, n) in cur] + [(t, 2 * n + 1) for (t, n) in cur]
            ords[(lvl, g)] = cur
    return ords


ORDS = _build_orders()

# x chunk schedule: (lvl, g, j, width, fp8)
XCHUNKS = []
for _g in range(G):
    for _l in range(DEPTH - 1, 4, -1):
        for _j in range(NCH_G[_l]):
            XCHUNKS.append((_l, _g, _j, min(CHUNK, GW[_l]), _l in FP8_LEVELS))
for _l in range(4, -1, -1):
    XCHUNKS.append((_l, None, 0, MW[_l], _l in FP8_LEVELS))
XIDX = {(l, g, j): i for i, (l, g, j, _, _) in enumerate(XCHUNKS)}
N8 = sum(1 for c in XCHUNKS if c[4])
N16 = len(XCHUNKS) - N8
X8IDX = {}
X16IDX = {}
for _l, _g, _j, _w, _f in XCHUNKS:
    if _f:
        X8IDX[(_l, _g, _j)] = len(X8IDX)
    else:
        X16IDX[(_l, _g, _j)] = len(X16IDX)


def build_program():
    nc = bacc.Bacc("TRN2", target_bir_lowering=False, debug=False)

    d_xt8 = nc.dram_tensor("xt8", [N8, 128, KX * CHUNK], FP8, kind="ExternalInput").ap()
    d_xt16 = nc.dram_tensor("xt16", [N16, 128, KX * CHUNK], BF16, kind="ExternalInput").ap()
    d_wioux = nc.dram_tensor("wioux", [KX, 128, 3 * H], BF16, kind="ExternalInput").ap()
    d_wiouh = nc.dram_tensor("wiouh", [KH, 128, 3 * H], BF16, kind="ExternalInput").ap()
    d_wfx = nc.dram_tensor("wfx", [KX, 128, H], BF16, kind="ExternalInput").ap()
    d_wfh = nc.dram_tensor("wfh", [KH, 128, H], BF16, kind="ExternalInput").ap()
    d_wioux8 = nc.dram_tensor("wioux8", [KX, 128, 3 * H], FP8, kind="ExternalInput").ap()
    d_wiouh8 = nc.dram_tensor("wiouh8", [KH, 128, 3 * H], FP8, kind="ExternalInput").ap()
    d_wfx8 = nc.dram_tensor("wfx8", [KX, 128, H], FP8, kind="ExternalInput").ap()
    d_wfh8 = nc.dram_tensor("wfh8", [KH, 128, H], FP8, kind="ExternalInput").ap()

    d_cout = nc.dram_tensor("c_out", [128, KH, BC], BF16, kind="ExternalOutput").ap()
    d_hout = nc.dram_tensor("h_out", [128, KH, BC], BF16, kind="ExternalOutput").ap()

    with tile.TileContext(nc) as tc, ExitStack() as ctx:
        wpool = ctx.enter_context(tc.tile_pool(name="weights", bufs=1))
        xpool = ctx.enter_context(tc.tile_pool(name="x", bufs=3))
        workp = ctx.enter_context(tc.tile_pool(name="work", bufs=2))
        statep = ctx.enter_context(tc.tile_pool(name="state", bufs=1))
        psump = ctx.enter_context(tc.tile_pool(name="psum", bufs=2, space="PSUM"))

        # ---- weights (fp8 leaf set first: needed immediately) ----
        s_wioux8 = wpool.tile([128, KX, 3 * H], FP8)
        s_wiouh8 = wpool.tile([128, KH, 3 * H], FP8)
        s_wfx8 = wpool.tile([128, KX, H], FP8)
        s_wfh8 = wpool.tile([128, KH, H], FP8)
        s_wioux = wpool.tile([128, KX, 3 * H], BF16)
        s_wiouh = wpool.tile([128, KH, 3 * H], BF16)
        s_wfx = wpool.tile([128, KX, H], BF16)
        s_wfh = wpool.tile([128, KH, H], BF16)
        for k in range(KX):
            nc.sync.dma_start(out=s_wioux8[:, k, :], in_=d_wioux8[k])
        for k in range(KH):
            nc.sync.dma_start(out=s_wiouh8[:, k, :], in_=d_wiouh8[k])
            nc.sync.dma_start(out=s_wfh8[:, k, :], in_=d_wfh8[k])
        for k in range(KX):
            nc.sync.dma_start(out=s_wfx8[:, k, :], in_=d_wfx8[k])
            nc.sync.dma_start(out=s_wioux[:, k, :], in_=d_wioux[k])
            nc.sync.dma_start(out=s_wfx[:, k, :], in_=d_wfx[k])
        for k in range(KH):
            nc.sync.dma_start(out=s_wiouh[:, k, :], in_=d_wiouh[k])
            nc.sync.dma_start(out=s_wfh[:, k, :], in_=d_wfh[k])

        # ---- state tile registry ----
        state = {}  # (lvl, g, j) -> (ht, ct, width)

        def state_tile(lvl, g, j, P, h_fp8):
            tag = f"st{lvl}"
            bufs = {8: 4, 7: 3, 6: 2, 5: 4}.get(lvl, 1)
            ht = statep.tile(
                [128, KH, P], FP8 if h_fp8 else BF16, tag=f"h{tag}", name=f"h{tag}", bufs=bufs
            )
            ct = statep.tile([128, KH, P], BF16, tag=f"c{tag}", name=f"c{tag}", bufs=bufs)
            state[(lvl, g, j)] = (ht, ct, P)
            return ht, ct

        def psum4(P=CHUNK):
            return psump.tile([128, KH, CHUNK], F32, tag="ps", name="ps")

        # ---- matmul emitters ----
        def mm_iou(ps, m0, xt, hsum, fp8, P, w8x, w16x, w8h, w16h):
            """Full iou accumulation (x K-chunks + optional h K-chunks) for
            output feature chunks m0..m0+3 into ps[:, m, :P].  Each (m, half)
            is a complete PSUM accumulation group."""
            for m in range(4):
                mm = 128 * (m0 + m)
                if fp8:
                    nh = (P + 255) // 256
                    for h in range(nh):
                        c0, cw = h * 256, min(256, P - h * 256)
                        nc.tensor.matmul(
                            ps[:, m, c0 : c0 + cw],
                            w8x[:, 0:2, mm : mm + 128],
                            xt[:, 0:2, c0 : c0 + cw],
                            start=True,
                            stop=False,
                            perf_mode=DR,
                        )
                        nc.tensor.matmul(
                            ps[:, m, c0 : c0 + cw],
                            w8x[:, 2, mm : mm + 128],
                            xt[:, 2, c0 : c0 + cw],
                            start=False,
                            stop=(hsum is None),
                        )
                        if hsum is not None:
                            for kp in range(2):
                                nc.tensor.matmul(
                                    ps[:, m, c0 : c0 + cw],
                                    w8h[:, 2 * kp : 2 * kp + 2, mm : mm + 128],
                                    hsum[:, 2 * kp : 2 * kp + 2, c0 : c0 + cw],
                                    start=False,
                                    stop=(kp == 1),
                                    perf_mode=DR,
                                )
                else:
                    for k in range(KX):
                        nc.tensor.matmul(
                            ps[:, m, :P],
                            w16x[:, k, mm : mm + 128],
                            xt[:, k, :P],
                            start=(k == 0),
                            stop=(hsum is None and k == KX - 1),
                        )
                    if hsum is not None:
                        for k in range(KH):
                            nc.tensor.matmul(
                                ps[:, m, :P],
                                w16h[:, k, mm : mm + 128],
                                hsum[:, k, :P],
                                start=False,
                                stop=(k == KH - 1),
                            )

        def mm_fx(ps, xt, fp8, P):
            """fx GEMM (K-chunks of x) into ps[:, m, :P], complete groups."""
            for m in range(4):
                mm = 128 * m
                if fp8:
                    nh = (P + 255) // 256
                    for h in range(nh):
                        c0, cw = h * 256, min(256, P - h * 256)
                        nc.tensor.matmul(
                            ps[:, m, c0 : c0 + cw],
                            s_wfx8[:, 0:2, mm : mm + 128],
                            xt[:, 0:2, c0 : c0 + cw],
                            start=True,
                            stop=False,
                            perf_mode=DR,
                        )
                        nc.tensor.matmul(
                            ps[:, m, c0 : c0 + cw],
                            s_wfx8[:, 2, mm : mm + 128],
                            xt[:, 2, c0 : c0 + cw],
                            start=False,
                            stop=True,
                        )
                else:
                    for k in range(KX):
                        nc.tensor.matmul(
                            ps[:, m, :P],
                            s_wfx[:, k, mm : mm + 128],
                            xt[:, k, :P],
                            start=(k == 0),
                            stop=(k == KX - 1),
                        )

        def mm_fh(ps, hs, fp8, P, dst0, src0):
            """fh GEMM over child cols hs[:, :, src0:src0+P] into
            ps[:, m, dst0:dst0+P], complete groups per (m, half)."""
            for m in range(4):
                mm = 128 * m
                if fp8:
                    nh = (P + 255) // 256
                    for h in range(nh):
                        c0, cw = h * 256, min(256, P - h * 256)
                        for kp in range(2):
                            nc.tensor.matmul(
                                ps[:, m, dst0 + c0 : dst0 + c0 + cw],
                                s_wfh8[:, 2 * kp : 2 * kp + 2, mm : mm + 128],
                                hs[:, 2 * kp : 2 * kp + 2, src0 + c0 : src0 + c0 + cw],
                                start=(kp == 0),
                                stop=(kp == 1),
                                perf_mode=DR,
                            )
                else:
                    for k in range(KH):
                        nc.tensor.matmul(
                            ps[:, m, dst0 : dst0 + P],
                            s_wfh[:, k, mm : mm + 128],
                            hs[:, k, src0 : src0 + P],
                            start=(k == 0),
                            stop=(k == KH - 1),
                        )

        def wide(tl, P):
            """[128, KH, P] -> flat [128, KH*P] AP view (strided ok)."""
            return tl[:, :, :P]

        # ---- generic chunk processors ----
        def process_split(lvl, g, j, P, fp8, pieces_l, pieces_r):
            """pieces_*: list of (child_key, src_off, dst_off, w); None=leaf."""
            xt = xpool.tile(
                [128, KX, CHUNK], FP8 if fp8 else BF16,
                tag="xt8" if fp8 else "xt16", name="xt8" if fp8 else "xt16",
            )
            if fp8:
                nc.sync.dma_start(
                    out=xt[:].rearrange("p k c -> p (k c)"), in_=d_xt8[X8IDX[(lvl, g, j)]]
                )
            else:
                nc.sync.dma_start(
                    out=xt[:].rearrange("p k c -> p (k c)"), in_=d_xt16[X16IDX[(lvl, g, j)]]
                )
            leaf = pieces_l is None

            hsum = None
            if not leaf:
                hsum = workp.tile(
                    [128, KH, CHUNK], FP8 if fp8 else BF16,
                    tag="hs8" if fp8 else "hs16", name="hs8" if fp8 else "hs16",
                )
                for (ckl, sl, dl, wl), (ckr, sr, _, _) in zip(pieces_l, pieces_r):
                    htl = state[ckl][0]
                    htr = state[ckr][0]
                    nc.vector.tensor_add(
                        out=hsum[:, :, dl : dl + wl],
                        in0=htl[:, :, sl : sl + wl],
                        in1=htr[:, :, sr : sr + wl],
                    )

            ht, ct = state_tile(lvl, g, j, P, lvl == 8 and (lvl - 1) in FP8_LEVELS)

            # u and i gates
            psU = psum4()
            mm_iou(psU, 8, xt, hsum, fp8, P, s_wioux8, s_wioux, s_wiouh8, s_wiouh)
            psI = psum4()
            mm_iou(psI, 0, xt, hsum, fp8, P, s_wioux8, s_wioux, s_wiouh8, s_wiouh)
            tu = workp.tile([128, KH, CHUNK], BF16, tag="tu")
            nc.scalar.activation(wide(tu, P), wide(psU, P), AFT.Tanh)
            si = workp.tile([128, KH, CHUNK], BF16, tag="si")
            nc.scalar.activation(wide(si, P), wide(psI, P), AFT.Sigmoid)
            nc.vector.tensor_mul(wide(ct, P), wide(si, P), wide(tu, P))

            # o gate + fx
            psO = psum4()
            mm_iou(psO, 4, xt, hsum, fp8, P, s_wioux8, s_wioux, s_wiouh8, s_wiouh)
            so = workp.tile([128, KH, CHUNK], BF16, tag="so", bufs=1)
            nc.scalar.activation(wide(so, P), wide(psO, P), AFT.Sigmoid)

            if not leaf:
                psF = psum4()
                mm_fx(psF, xt, fp8, P)
                fx = workp.tile([128, KH, CHUNK], BF16, tag="fx", bufs=1)
                nc.scalar.activation(wide(fx, P), wide(psF, P), AFT.Copy)

                # forget gates + fc per side
                for pieces in (pieces_l, pieces_r):
                    psH = psum4()
                    for ck, so_, do_, w_ in pieces:
                        chh = state[ck][0]
                        mm_fh(psH, chh, fp8, w_, do_, so_)
                    f = workp.tile([128, KH, CHUNK], BF16, tag="f")
                    nc.vector.scalar_tensor_tensor(
                        out=wide(f, P),
                        in0=wide(psH, P),
                        scalar=1.0,
                        in1=wide(fx, P),
                        op0=mybir.AluOpType.mult,
                        op1=mybir.AluOpType.add,
                    )
                    nc.scalar.activation(wide(f, P), wide(f, P), AFT.Sigmoid)
                    fc = workp.tile([128, KH, CHUNK], BF16, tag="fc")
                    for ck, so_, do_, w_ in pieces:
                        chc = state[ck][1]
                        nc.vector.tensor_mul(
                            fc[:, :, do_ : do_ + w_],
                            f[:, :, do_ : do_ + w_],
                            chc[:, :, so_ : so_ + w_],
                        )
                    nc.vector.tensor_add(wide(ct, P), wide(ct, P), wide(fc, P))

            # h = sig(o) * tanh(c)
            tc_ = workp.tile([128, KH, CHUNK], BF16, tag="tu")
            nc.scalar.activation(wide(tc_, P), wide(ct, P), AFT.Tanh)
            nc.vector.tensor_mul(wide(ht, P), wide(so, P), wide(tc_, P))

            if lvl == 0:
                nc.sync.dma_start(out=d_hout, in_=ht[:, :, :P])
                nc.sync.dma_start(out=d_cout, in_=ct[:, :, :P])

        def process_interleaved(lvl, P):
            """Small merged level: children at (lvl+1, None, 0), interleaved
            stride-2.  fx folded into the fh PSUM via 0-step dup AP."""
            xt = xpool.tile([128, KX, CHUNK], BF16, tag="xt16", name="xt16")
            nc.sync.dma_start(
                out=xt[:].rearrange("p k c -> p (k c)"), in_=d_xt16[X16IDX[(lvl, None, 0)]]
            )
            chh, chc, cw = state[(lvl + 1, None, 0)]
            C2 = 2 * P

            pair_h = chh[:, :, :C2].rearrange("p k (n two) -> p k n two", two=2)
            hsum = workp.tile([128, KH, CHUNK], BF16, tag="hs16", name="hs16")
            nc.vector.tensor_add(
                out=hsum[:, :, :P], in0=pair_h[:, :, :, 0], in1=pair_h[:, :, :, 1]
            )

            ht, ct = state_tile(lvl, None, 0, P, False)

            psU = psum4()
            mm_iou(psU, 8, xt, hsum, False, P, None, s_wioux, None, s_wiouh)
            psI = psum4()
            mm_iou(psI, 0, xt, hsum, False, P, None, s_wioux, None, s_wiouh)
            tu = workp.tile([128, KH, CHUNK], BF16, tag="tu")
            nc.scalar.activation(wide(tu, P), wide(psU, P), AFT.Tanh)
            si = workp.tile([128, KH, CHUNK], BF16, tag="si")
            nc.scalar.activation(wide(si, P), wide(psI, P), AFT.Sigmoid)
            nc.vector.tensor_mul(wide(ct, P), wide(si, P), wide(tu, P))

            psO = psum4()
            mm_iou(psO, 4, xt, hsum, False, P, None, s_wioux, None, s_wiouh)
            so = workp.tile([128, KH, CHUNK], BF16, tag="so", bufs=1)
            nc.scalar.activation(wide(so, P), wide(psO, P), AFT.Sigmoid)

            # f over interleaved child cols; fx dup'd via 0-step AP
            psH = psum4()
            for m in range(4):
                mm = 128 * m
                for k in range(KH):
                    nc.tensor.matmul(
                        psH[:, m, :C2],
                        s_wfh[:, k, mm : mm + 128],
                        chh[:, k, :C2],
                        start=(k == 0),
                        stop=False,
                    )
                for k in range(KX):
                    xb = xt[:, k, :P]
                    xdup = bass.AP(
                        tensor=xb.tensor, offset=xb.offset, ap=list(xb.ap) + [[0, 2]]
                    )
                    nc.tensor.matmul(
                        psH[:, m, :C2],
                        s_wfx[:, k, mm : mm + 128],
                        xdup,
                        start=False,
                        stop=(k == KX - 1),
                    )
            f = workp.tile([128, KH, CHUNK], BF16, tag="f")
            nc.scalar.activation(wide(f, C2), wide(psH, C2), AFT.Sigmoid)
            fc = workp.tile([128, KH, CHUNK], BF16, tag="fc")
            nc.vector.tensor_mul(wide(fc, C2), wide(f, C2), chc[:, :, :C2])
            pair_fc = fc[:, :, :C2].rearrange("p k (n two) -> p k n two", two=2)
            nc.vector.tensor_add(wide(ct, P), wide(ct, P), pair_fc[:, :, :, 0])
            nc.vector.tensor_add(wide(ct, P), wide(ct, P), pair_fc[:, :, :, 1])

            tc_ = workp.tile([128, KH, CHUNK], BF16, tag="tu")
            nc.scalar.activation(wide(tc_, P), wide(ct, P), AFT.Tanh)
            nc.vector.tensor_mul(wide(ht, P), wide(so, P), wide(tc_, P))

            if lvl == 0:
                nc.sync.dma_start(out=d_hout, in_=ht[:, :, :P])
                nc.sync.dma_start(out=d_cout, in_=ct[:, :, :P])

        # ---- schedule ----
        for g in range(G):
            for j in range(NCH_G[8]):
                process_split(8, g, j, CHUNK, 8 in FP8_LEVELS, None, None)
            for j in range(NCH_G[7]):
                process_split(
                    7, g, j, CHUNK, 7 in FP8_LEVELS,
                    [((8, g, j), 0, 0, CHUNK)],
                    [((8, g, 2 + j), 0, 0, CHUNK)],
                )
            process_split(
                6, g, 0, CHUNK, 6 in FP8_LEVELS,
                [((7, g, 0), 0, 0, CHUNK)],
                [((7, g, 1), 0, 0, CHUNK)],
            )
            process_split(
                5, g, 0, GW[5], 5 in FP8_LEVELS,
                [((6, g, 0), 0, 0, GW[5])],
                [((6, g, 0), GW[5], 0, GW[5])],
            )
        process_split(
            4, None, 0, MW[4], False,
            [((5, g, 0), 0, 128 * g, 128) for g in range(G)],
            [((5, g, 0), 128, 128 * g, 128) for g in range(G)],
        )
        for lvl in range(3, -1, -1):
            process_interleaved(lvl, MW[lvl])

    nc.compile()
    return nc


_nc_cache = None


def get_program():
    global _nc_cache
    if _nc_cache is None:
        _nc_cache = build_program()
    return _nc_cache


def prep_inputs(inputs, W_ioux, b_ioux, W_iouh, b_iouh, W_fx, b_fx, W_fh, b_fh):
    """Host-side prep: per-core x^T slabs (split/interleaved orders, constant-1
    bias rows) + weight chunks with bias rows, in bf16 and e4m3."""
    import ml_dtypes

    BF = ml_dtypes.bfloat16
    E4 = ml_dtypes.float8_e4m3

    inputs = np.ascontiguousarray(np.asarray(inputs, dtype=np.float32))

    def pack_w(w, bias, rows):
        """[K, M] weights + bias -> bf16 [rows,128,M] and e4m3 with the bias
        split hi/lo across rows 300/301 (fp8 rounding compensation)."""
        w = np.asarray(w, np.float32)
        m = w.shape[1]
        full = np.zeros((rows * 128, m), np.float32)
        full[: w.shape[0]] = w
        if bias is not None:
            full[DIN] = np.asarray(bias, np.float32)
        w16 = np.ascontiguousarray(full.reshape(rows, 128, m).astype(BF))
        f8 = full.copy()
        if bias is not None:
            bhi = np.asarray(bias, np.float32).astype(E4).astype(np.float32)
            f8[DIN] = bhi
            f8[DIN + 1] = np.asarray(bias, np.float32) - bhi
        w8 = np.ascontiguousarray(f8.reshape(rows, 128, m).astype(E4))
        return w16, w8

    wioux, wioux8 = pack_w(W_ioux, np.asarray(b_ioux) + np.asarray(b_iouh), KX)
    wiouh, wiouh8 = pack_w(W_iouh, None, KH)
    wfx, wfx8 = pack_w(W_fx, np.asarray(b_fx) + np.asarray(b_fh), KX)
    wfh, wfh8 = pack_w(W_fh, None, KH)

    in_maps = []
    for c in range(NCORES):
        xc = inputs[c * BC : (c + 1) * BC]  # [BC, NTREE, DIN]
        xt8 = np.zeros((N8, 128, KX, CHUNK), np.float32)
        xt16 = np.zeros((N16, 128, KX, CHUNK), np.float32)
        for lvl, g, j, w, fp8 in XCHUNKS:
            cols = ORDS[(lvl, g)][j * CHUNK : j * CHUNK + w]
            xcols = np.stack(
                [xc[t, (1 << lvl) - 1 + n, :] for (t, n) in cols], axis=1
            )  # [DIN, w]
            blk = np.zeros((KX * 128, CHUNK), np.float32)
            blk[:DIN, :w] = xcols
            blk[DIN, :w] = 1.0
            blk[DIN + 1, :w] = 1.0
            blk = blk.reshape(KX, 128, CHUNK).transpose(1, 0, 2)
            if fp8:
                xt8[X8IDX[(lvl, g, j)]] = blk
            else:
                xt16[X16IDX[(lvl, g, j)]] = blk
        in_maps.append(
            {
                "xt8": np.ascontiguousarray(xt8.reshape(N8, 128, KX * CHUNK).astype(E4)),
                "xt16": np.ascontiguousarray(
                    xt16.reshape(N16, 128, KX * CHUNK).astype(BF)
                ),
                "wioux": wioux,
                "wiouh": wiouh,
                "wfx": wfx,
                "wfh": wfh,
                "wioux8": wioux8,
                "wiouh8": wiouh8,
                "wfx8": wfx8,
                "wfh8": wfh8,
            }
        )
    return in_maps


def assemble_output(results):
    """results: list of per-core dicts with c_out/h_out [128, KH, BC] bf16."""
    cs, hs = [], []
    for r in results:
        c = np.transpose(np.asarray(r["c_out"], np.float32), (2, 1, 0)).reshape(BC, H)
        h = np.transpose(np.asarray(r["h_out"], np.float32), (2, 1, 0)).reshape(BC, H)
        cs.append(c)
        hs.append(h)
    return np.concatenate(cs, 0), np.concatenate(hs, 0)


def run_on_hw(in_maps, trace=False, tmpdir=None):
    from concourse.bass_utils import run_bass_kernel_spmd

    nc = get_program()
    return run_bass_kernel_spmd(
        nc, in_maps, list(range(NCORES)), trace=trace, tmpdir=tmpdir
    )


def kernel(**inputs):
    in_maps = prep_inputs(**inputs)
    res = run_on_hw(in_maps)
    return assemble_output(res.results)
